# revision 11
# baseline (speedup 1.0000x reference)
"""Constraint-projection layer on 8 Trainium2 NeuronCores.

Reference computes, per batch row y_i:  x_i = argmin ||x - y_i|| s.t. A x = b_i
via a dense KKT solve. Closed form (Schur complement of the KKT system):

    x = y - A^T (A A^T)^{-1} (A y - b)

Host precomputes W = (A A^T)^{-1} A  (128 x 1024, float64 solve, cast f32).
Each core gets a 2048-row batch shard in TRANSPOSED layout (dim-major), so
both matmuls contract over the partition axis with contiguous DMA only:

    stage 1:  T^T = A @ Y^T - B^T          (128 m  x 2048 batch)
    stage 2:  X^T = Y^T - W_chunk^T @ T^T  (1024 d x 2048 batch)

Data-parallel: no cross-core communication.
"""

import os

import numpy as np
import bass_rust as _br
import concourse.bass as bass
import concourse.mybir as mybir
from concourse import tile
from concourse.bass_utils import run_bass_kernel_spmd

F32 = mybir.dt.float32
F32R = mybir.dt.float32r
# fp32r streams through the PE at 4x the fp32 rate (1 cycle/row vs 4).
USE_F32R = os.environ.get("KERNEL_F32R", "0") == "1"


def _mm_ap(ap):
    return ap.bitcast(F32R) if USE_F32R else ap

N_CORES = 8
BATCH = 16384
N = 1024           # input dim
M = 128            # constraint dim
BC = BATCH // N_CORES  # 2048 batch rows per core
KC = N // 128      # 8 contraction chunks
F = 512            # free-dim tile (one PSUM bank of f32)
NJ = BC // F       # 4 batch tiles per core


def _split_drain_and_barrier(self, tick_clock, wait_clock):
    # Walrus in this toolchain rejects >2 sync waits on the Tile tail Drain
    # (CTRL_NO_STRUCT). Emit one-wait-per-nop instructions ahead of the
    # drain instead; sequentially identical on the sync sequencer.
    gc = tick_clock.global_clock
    vals = eval(repr(gc).replace("VectorClock", "").strip("()"))
    for i, v in enumerate(vals):
        if v:
            single = [0] * len(vals)
            single[i] = v
            nop = self.nc.sync.nop(nofuse=True)
            wait_clock.add_sem_waits(
                nop.ins, _br.ScopedClock({None: _br.VectorClock(single)})
            )
    self.nc.sync.drain()
    self.nc.all_engine_barrier()
    assert self.sems is not None
    popped = self.nc._tile_sem_poison_stack.pop()
    assert popped is self._sem_poison
    self.nc.clear_and_free_semaphores(list(self.sems.allocated().values()))
    self.nc.all_engine_barrier()


tile.TileContext._drain_and_barrier = _split_drain_and_barrier

_orig_commit_and_lower = tile.TileContext._commit_and_lower

# Same walrus limitation for regular instructions: Matmult (S3_LW) takes no
# extra sync waits, most others take one. Spill excess waits onto dedicated
# same-engine nops committed immediately before the instruction.
_ZERO_WAIT_OPS = ("InstMatmult", "InstDrain")


def _split_commit_and_lower(self, inst, original_block, old_bb_map, bb_to_exit_bb):
    tn = type(inst).__name__
    if tn.startswith("Inst") and inst.engine is not None:
        si = inst.sync_info
        if si is not None:
            waits = list(si.on_wait)
            keep = 0 if tn in _ZERO_WAIT_OPS else 1
            if len(waits) > keep:
                spill, keep_waits = (
                    (waits, []) if keep == 0 else (waits[:-1], [waits[-1]])
                )
                for w_ in spill:
                    nop = mybir.InstNoOp(
                        name=self.nc.get_next_instruction_name(),
                        engine=inst.engine,
                        sync_info=mybir.SyncInfo(on_wait=[w_], on_update=[]),
                        bass_nofuse=True,
                    )
                    self._commit_instruction(nop)
                inst.sync_info = mybir.SyncInfo(
                    on_wait=keep_waits, on_update=list(si.on_update)
                )
    return _orig_commit_and_lower(self, inst, original_block, old_bb_map, bb_to_exit_bb)


tile.TileContext._commit_and_lower = _split_commit_and_lower


def build_nc() -> bass.Bass:
    nc = bass.Bass()
    yt_d = nc.declare_dram_parameter("yt", [N, BC], F32, isOutput=False)
    bt_d = nc.declare_dram_parameter("bt", [M, BC], F32, isOutput=False)
    at_d = nc.declare_dram_parameter("at", [N, M], F32, isOutput=False)
    w_d = nc.declare_dram_parameter("w", [M, N], F32, isOutput=False)
    out_d = nc.declare_dram_parameter("out", [N, BC], F32, isOutput=True)

    # dim-chunked 3D views: partition = row-within-chunk, then (chunk, batch)
    yt_v = yt_d.rearrange("(k p) b -> p k b", p=128)
    at_v = at_d.rearrange("(k p) m -> p k m", p=128)
    out_v = out_d.rearrange("(k p) b -> p k b", p=128)

    with tile.TileContext(nc) as tc:
        with (
            tc.tile_pool(name="const", bufs=1) as constp,
            tc.tile_pool(name="yts", bufs=3) as ytp,
            tc.tile_pool(name="tts", bufs=2) as ttp,
            tc.tile_pool(name="outs", bufs=2) as outp,
            tc.tile_pool(name="ps1", bufs=2, space="PSUM") as ps1,
            tc.tile_pool(name="ps2", bufs=4, space="PSUM") as ps2,
        ):
            at_s = constp.tile([128, KC, M], F32)  # A^T chunks: p=dim, free=m
            nc.sync.dma_start(_mm_ap(at_s[:]), _mm_ap(at_v[:]))
            w_s = constp.tile([128, N], F32)  # partition = m, free = dim
            nc.sync.dma_start(_mm_ap(w_s[:]), _mm_ap(w_d[:]))
            bt_s = constp.tile([128, BC], F32)  # partition = m, free = batch
            nc.sync.dma_start(bt_s[:], bt_d[:])

            # j-major software pipeline: each batch tile of 512 flows
            # load -> mm1(accum 8) -> sub -> 8x(mm2 -> sub) -> store
            # independently, so input DMA, PE, DVE, and output DMA overlap.
            for j in range(NJ):
                ytj = ytp.tile([128, KC, F], F32)
                nc.sync.dma_start(
                    _mm_ap(ytj[:]), _mm_ap(yt_v[:, :, j * F:(j + 1) * F])
                )

                pt = ps1.tile([128, F], F32)
                for k in range(KC):
                    nc.tensor.matmul(
                        pt[:],
                        _mm_ap(at_s[:, k, :]),
                        _mm_ap(ytj[:, k, :]),
                        start=(k == 0),
                        stop=(k == KC - 1),
                    )
                tt = ttp.tile([128, F], F32)
                nc.vector.tensor_sub(
                    _mm_ap(tt[:]), pt[:], bt_s[:, j * F:(j + 1) * F]
                )

                oj = outp.tile([128, KC, F], F32)
                for d in range(KC):
                    p2 = ps2.tile([128, F], F32)
                    nc.tensor.matmul(
                        p2[:],
                        _mm_ap(w_s[:, d * 128:(d + 1) * 128]),
                        _mm_ap(tt[:]),
                        start=True,
                        stop=True,
                    )
                    nc.vector.tensor_sub(oj[:, d, :], ytj[:, d, :], p2[:])
                nc.sync.dma_start(out_v[:, :, j * F:(j + 1) * F], oj[:])
    return nc


_NC_CACHE = None
_RUNNER = None


def _get_nc():
    global _NC_CACHE
    if _NC_CACHE is None:
        _NC_CACHE = build_nc()
    return _NC_CACHE


def _build_runner():
    """Persistent jitted shard_map callable over 8 cores (mirrors
    bass2jax.run_bass_via_pjrt's multi-core path, but cached so repeated
    kernel() calls skip retracing/XLA recompile)."""
    import jax
    from jax.sharding import Mesh, PartitionSpec
    from jax.experimental.shard_map import shard_map
    from concourse import bass2jax as b2j

    nc = _get_nc()
    b2j.install_neuronx_cc_hook()
    assert nc.dbg_addr is None
    partition_name = nc.partition_id_tensor.name if nc.partition_id_tensor else None

    in_names, out_names, out_avals, zero_shapes = [], [], [], []
    for alloc in nc.m.functions[0].allocations:
        if not isinstance(alloc, mybir.MemoryLocationSet):
            continue
        name = alloc.memorylocations[0].name
        if alloc.kind == "ExternalInput":
            if name != partition_name:
                in_names.append(name)
        elif alloc.kind == "ExternalOutput":
            out_names.append(name)
            shape = tuple(alloc.tensor_shape)
            dtype = mybir.dt.np(alloc.dtype)
            out_avals.append(jax.core.ShapedArray(shape, dtype))
            zero_shapes.append((shape, dtype))
    n_params = len(in_names)
    n_outs = len(out_names)
    all_in_names = tuple(in_names) + tuple(out_names)
    if partition_name is not None:
        all_in_names = all_in_names + (partition_name,)

    def _body(*args):
        operands = list(args)
        if partition_name is not None:
            operands.append(b2j.partition_id_tensor())
        outs = b2j._bass_exec_p.bind(
            *operands,
            out_avals=tuple(out_avals),
            in_names=all_in_names,
            out_names=tuple(out_names),
            lowering_input_output_aliases=(),
            sim_require_finite=True,
            sim_require_nnan=True,
            nc=nc,
        )
        return tuple(outs)

    devices = jax.devices()[:N_CORES]
    mesh = Mesh(np.asarray(devices), ("core",))
    in_specs = (PartitionSpec("core"),) * (n_params + n_outs)
    out_specs = (PartitionSpec("core"),) * n_outs
    donate = tuple(range(n_params, n_params + n_outs))
    sharded = jax.jit(
        shard_map(
            _body, mesh=mesh, in_specs=in_specs, out_specs=out_specs,
            check_rep=False,
        ),
        donate_argnums=donate,
        keep_unused=True,
    )

    from jax.sharding import NamedSharding

    zeros_fns = [
        jax.jit(
            lambda s=shape, d=dtype: jax.numpy.zeros(
                (N_CORES * s[0], *s[1:]), d
            ),
            out_shardings=NamedSharding(mesh, PartitionSpec("core")),
        )
        for shape, dtype in zero_shapes
    ]

    def run(named_inputs: dict):
        """named_inputs: name -> concatenated (N_CORES*dim0, ...) array."""
        ins = [named_inputs[n] for n in in_names]
        zeros = [f() for f in zeros_fns]
        outs = sharded(*ins, *zeros)
        return dict(zip(out_names, outs))

    run._parts = {
        "sharded": sharded,
        "in_names": in_names,
        "out_names": out_names,
        "mesh": mesh,
        "zeros_fns": zeros_fns,
    }
    return run


def _get_runner():
    global _RUNNER
    if _RUNNER is None:
        _RUNNER = _build_runner()
    return _RUNNER


def _prep_inputs(y, A, b):
    A64 = A.astype(np.float64)
    W = np.linalg.solve(A64 @ A64.T, A64).astype(np.float32)  # (M, N)
    AT = np.ascontiguousarray(A.T)  # (N, M)
    # concat-over-cores layouts expected by the shard_map runner
    yt_cat = np.ascontiguousarray(
        y.reshape(N_CORES, BC, N).transpose(0, 2, 1)
    ).reshape(N_CORES * N, BC)
    bt_cat = np.ascontiguousarray(
        b.reshape(N_CORES, BC, M).transpose(0, 2, 1)
    ).reshape(N_CORES * M, BC)
    at_cat = np.broadcast_to(AT, (N_CORES, N, M)).reshape(N_CORES * N, M)
    w_cat = np.broadcast_to(W, (N_CORES, M, N)).reshape(N_CORES * M, N)
    return {"yt": yt_cat, "bt": bt_cat, "at": at_cat, "w": w_cat}


def _unpack_output(out_cat: np.ndarray) -> np.ndarray:
    return np.ascontiguousarray(
        np.asarray(out_cat).reshape(N_CORES, N, BC).transpose(0, 2, 1)
    ).reshape(BATCH, N)


def kernel(y: np.ndarray, A: np.ndarray, b: np.ndarray) -> np.ndarray:
    y = np.ascontiguousarray(np.asarray(y, dtype=np.float32))
    A = np.ascontiguousarray(np.asarray(A, dtype=np.float32))
    b = np.ascontiguousarray(np.asarray(b, dtype=np.float32))
    assert y.shape == (BATCH, N) and A.shape == (M, N) and b.shape == (BATCH, M)

    named = _prep_inputs(y, A, b)
    try:
        run = _get_runner()
        out = run(named)["out"]
        return _unpack_output(out)
    except Exception:
        # Fallback: slower but uses only the public SPMD entry point.
        in_maps = [
            {
                k: np.ascontiguousarray(
                    v.reshape(N_CORES, v.shape[0] // N_CORES, *v.shape[1:])[i]
                )
                for k, v in named.items()
            }
            for i in range(N_CORES)
        ]
        res = run_bass_kernel_spmd(_get_nc(), in_maps, list(range(N_CORES)))
        x = np.empty((BATCH, N), dtype=np.float32)
        for i in range(N_CORES):
            x[i * BC:(i + 1) * BC, :] = res.results[i]["out"].T
        return x
